# revision 3
# baseline (speedup 1.0000x reference)
"""Multi-head attention (B=2, S=2048, D=1024, H=16) on 8 Trainium2 cores.

Sharding: batch x head-group. Core c handles batch b = c//4 and heads
4*(c%4) .. 4*(c%4)+4 (a 256-wide slice of the feature dim).

Structure (single fused pass per core):
  - Q then K projections (transposed [d, s] outputs, f16), matmuls chasing
    the x-tile DMA stream; weights batched in single rearranged DMAs on the
    ACT queue so the SP queue only carries x.
  - attention per (512-wide q chunk, head pair): scores [k, q] on PE (f16,
    two heads row-packed), exp on ScalarE (the pace-setting engine), attn@V
    accumulated in PSUM with a ones-column emitting the softmax denominator.
  - v-projection chunks and the output projection are interleaved into the
    attention loop's PE slack (v chunks inside unit 0, fin blocks dripped
    into later units); y written per s-tile.
All attention operands are f16 (FWL-eligible matmuls, half SBUF traffic);
accumulation stays fp32 in PSUM.
"""
import copy
import sys

import numpy as np

if "/opt/trn_rl_repo" not in sys.path:
    sys.path.insert(0, "/opt/trn_rl_repo")

B = 2
S = 2048
DIM = 1024
NH = 16
HD = 64
NCORES = 8
GROUPS = NCORES // B          # 4 head-groups per batch
HPC = NH // GROUPS            # 4 heads per core
CS = HPC * HD                 # 256-wide feature slice per core
PAIRS = HPC // 2              # head pairs per core


def _split_waits(nc, templates, max_waits=1):
    """This walrus build rejects instructions carrying more than one sync-wait
    command. Move excess waits onto injected same-engine NOPs placed right
    before the over-subscribed instruction."""
    from concourse import mybir

    n_nops = 0
    for f in nc.m.functions:
        for blk in f.blocks:
            insts = blk.instructions
            i = 0
            while i < len(insts):
                inst = insts[i]
                si = inst.sync_info
                if si is not None and si.on_wait and len(si.on_wait) > max_waits:
                    waits = list(si.on_wait)
                    keep = waits[-max_waits:]
                    extra = waits[:-max_waits]
                    nops = []
                    for w in extra:
                        nop = copy.deepcopy(templates[inst.engine])
                        nop.name = f"waitnop-{n_nops}"
                        nop.sync_info = mybir.SyncInfo(on_wait=[w], on_update=[])
                        nops.append(nop)
                        n_nops += 1
                    inst.sync_info = mybir.SyncInfo(
                        on_wait=keep, on_update=list(si.on_update))
                    insts[i:i] = nops
                    i += len(nops)
                i += 1
    return n_nops

_PROGRAM = None


def _build_program(split=True, phases=('qk', 'v', 'att', 'fin'), loop_n=1):
    import concourse.bass as bass
    import concourse.tile as tile
    from concourse import mybir
    import contextlib

    dt = mybir.dt
    f32 = dt.float32
    f16 = dt.float16
    ACT = mybir.ActivationFunctionType

    nc = bass.Bass()
    nop_templates = {
        eng.engine: eng.nop().ins
        for eng in (nc.tensor, nc.vector, nc.scalar, nc.gpsimd, nc.sync)
    }

    xqT = nc.dram_tensor("xqT", [DIM, S], f16, kind="ExternalInput")
    xkT = nc.dram_tensor("xkT", [DIM, S], f16, kind="ExternalInput")
    xvT = nc.dram_tensor("xvT", [DIM, S], f16, kind="ExternalInput")
    wqT = nc.dram_tensor("wqT", [DIM, CS], f16, kind="ExternalInput")
    wkT = nc.dram_tensor("wkT", [DIM, CS], f16, kind="ExternalInput")
    wvT = nc.dram_tensor("wvT", [DIM, CS], f16, kind="ExternalInput")
    woT = nc.dram_tensor("woT", [CS, DIM], f16, kind="ExternalInput")
    bq_s = nc.dram_tensor("bq_s", [CS, 1], f32, kind="ExternalInput")
    bk_s = nc.dram_tensor("bk_s", [CS, 1], f32, kind="ExternalInput")
    bv_s = nc.dram_tensor("bv_s", [1, CS], f32, kind="ExternalInput")
    ones_c = nc.dram_tensor("ones_c", [1, (S // 128) * HPC], f16, kind="ExternalInput")
    y = nc.dram_tensor("y", [S, DIM], f32, kind="ExternalOutput")

    KT_PROJ = DIM // 128
    KT_ATT = S // 128
    ST = S // 128
    SCALE = 1.0 / np.sqrt(np.float32(DIM))

    with tile.TileContext(nc) as tc:
        loop_cm = (
            tc.For_i(0, loop_n, 1, hint_engines=(
                mybir.EngineType.PE, mybir.EngineType.Activation,
                mybir.EngineType.DVE, mybir.EngineType.SP))
            if loop_n > 1 else contextlib.nullcontext())
        with (
            loop_cm,
            tc.tile_pool(name="weights", bufs=1) as wpool,
            tc.tile_pool(name="persist", bufs=1) as persist,
            tc.tile_pool(name="xstream", bufs=4) as xpool,
            tc.tile_pool(name="xvstream", bufs=3) as xvpool,
            tc.tile_pool(name="exp", bufs=3) as expool,
            tc.tile_pool(name="small", bufs=2) as spool,
            tc.tile_pool(name="yout", bufs=2) as ypool,
        ):
            # ---- SBUF allocations ----
            wq_sb = wpool.tile([128, KT_PROJ, CS], f16, tag="wq")
            wk_sb = wpool.tile([128, KT_PROJ, CS], f16, tag="wk")
            wv_sb = wpool.tile([128, KT_PROJ, CS], f16, tag="wv")
            wo_sb = wpool.tile([128, PAIRS, DIM], f16, tag="wo")
            bq_sb = wpool.tile([128, PAIRS], f32, tag="bq")
            bk_sb = wpool.tile([128, PAIRS], f32, tag="bk")
            vb_sb = wpool.tile([128, CS], f32, tag="vb")
            qhT = persist.tile([128, PAIRS, S], f16, tag="qhT")
            khT = persist.tile([128, PAIRS, S], f16, tag="khT")
            vh = persist.tile([128, ST, HPC, HD + 1], f16, tag="vh")
            out_sT = persist.tile([128, PAIRS, S], f16, tag="out_sT")

            # ---- prologue DMAs: weights batched on the ACT queue so the SP
            # queue only carries the x streams; xv late (consumed during att).
            nc.scalar.dma_start(wq_sb[:], wqT.rearrange("(a p) c -> p a c", p=128))
            nc.scalar.dma_start(wk_sb[:], wkT.rearrange("(a p) c -> p a c", p=128))
            for pr in range(PAIRS):
                nc.scalar.dma_start(bq_sb[:, pr:pr + 1], bq_s[pr * 128:(pr + 1) * 128, :])
                nc.scalar.dma_start(bk_sb[:, pr:pr + 1], bk_s[pr * 128:(pr + 1) * 128, :])
            nc.scalar.dma_start(wv_sb[:], wvT.rearrange("(a p) c -> p a c", p=128))
            nc.scalar.dma_start(vb_sb[:], bv_s[:].to_broadcast((128, CS)))
            nc.scalar.dma_start(
                vh[:, :, :, HD:HD + 1],
                ones_c[:].to_broadcast((128, ST * HPC)))
            nc.scalar.dma_start(wo_sb[:], woT.rearrange("(a p) c -> p a c", p=128))

            if 'qk' in phases:
                # ---- Q/K projections (transposed outputs, head-pair layout),
                # matmuls chase the x tiles; dedicated buffers so every x DMA
                # issues immediately ----
                for (xT, w_sb, out_sb, bias_sb, xtag, xbufs) in (
                    (xqT, wq_sb, qhT, bq_sb, "xq", 8),
                    (xkT, wk_sb, khT, bk_sb, "xk", 7),
                ):
                    with tc.tile_pool(name="qkpsum", bufs=2, space="PSUM") as qkp:
                        ps = [qkp.tile([128, S], f32, tag="qk", name=f"qkps{i}") for i in range(PAIRS)]
                        for kt in range(KT_PROJ):
                            xt = xpool.tile([128, S], f16, tag=xtag,
                                            name=f"{xtag}{kt}", bufs=xbufs)
                            nc.sync.dma_start(xt[:], xT[kt * 128:(kt + 1) * 128, :])
                            for pr in range(PAIRS):
                                for qc in range(S // 512):
                                    nc.tensor.matmul(
                                        ps[pr][:, qc * 512:(qc + 1) * 512],
                                        w_sb[:, kt, pr * 128:(pr + 1) * 128],
                                        xt[:, qc * 512:(qc + 1) * 512],
                                        start=(kt == 0), stop=(kt == KT_PROJ - 1),
                                    )
                        for pr in range(PAIRS):
                            nc.scalar.activation(
                                out_sb[:, pr, :], ps[pr][:],
                                ACT.Identity, bias=bias_sb[:, pr:pr + 1], scale=1.0)

            # ---- V projection + attention + output projection, fully
            # interleaved. One PSUM pool: sc (2 banks x2) + at (1 bank x2) +
            # vp/yp (1 bank x2) = 8 banks.
            vap = tc.alloc_tile_pool(name="vattps", bufs=2, space="PSUM")
            dnrpool = tc.alloc_tile_pool(name="dnr", bufs=2, space="DRAM")

            if 'v' in phases:
                # xv slices stream on the SP queue behind xq/xk; v-projection
                # chunk st is emitted inside attention unit 0's kt loop just
                # before the attnV that consumes it.
                xvT_r = xvT.rearrange("(t p) (s c) -> p t s c", p=128, c=256)
                xv_tiles = []
                for st2 in range(ST // 2):
                    xvt = xvpool.tile([128, KT_PROJ, 256], f16, tag="xv",
                                      name=f"xv{st2}", bufs=8)
                    nc.sync.dma_start(xvt[:], xvT_r[:, :, st2, :])
                    xv_tiles.append(xvt)

                def emit_vchunk(st):
                    st2, sub = st // 2, st % 2
                    vp = vap.tile([128, CS], f32, tag="vp", name="vp")
                    for kt2 in range(KT_PROJ):
                        nc.tensor.matmul(
                            vp[:], xv_tiles[st2][:, kt2, sub * 128:(sub + 1) * 128],
                            wv_sb[:, kt2, :],
                            start=(kt2 == 0), stop=(kt2 == KT_PROJ - 1))
                    nc.vector.tensor_add(
                        vh[:, st, :, 0:HD],
                        vp[:].rearrange("p (h c) -> p h c", c=HD),
                        vb_sb[:].rearrange("p (h c) -> p h c", c=HD))

            if 'att' in phases:
                units = [(qck, pr) for qck in range(S // 512)
                         for pr in range(PAIRS)]
                fin_queue = []          # (st, n2) output-projection blocks
                ysb_cur = {}

                def emit_fin_block():
                    if not fin_queue or 'fin' not in phases:
                        return
                    st, n2 = fin_queue.pop(0)
                    if n2 == 0:
                        ysb_cur[st] = ypool.tile([128, DIM], f32, tag="ysb",
                                                 name=f"ysb{st}")
                    yp = vap.tile([128, 512], f32, tag="vp", name="yp")
                    for pr2 in range(PAIRS):
                        nc.tensor.matmul(
                            yp[:],
                            out_sT[:, pr2, st * 128:(st + 1) * 128],
                            wo_sb[:, pr2, n2 * 512:(n2 + 1) * 512],
                            start=(pr2 == 0), stop=(pr2 == PAIRS - 1))
                    nc.vector.tensor_copy(
                        ysb_cur[st][:, n2 * 512:(n2 + 1) * 512], yp[:])
                    if n2 == 1 and 'noy' not in phases:
                        nc.sync.dma_start(
                            y[st * 128:(st + 1) * 128, :], ysb_cur[st][:])

                def mk_scores(pr, q0):
                    def scores(kt):
                        sct = vap.tile([128, 1024], f32, tag="sc",
                                       name="sct")
                        for h in range(2):
                            nc.tensor.matmul(
                                sct[:, h * 512:(h + 1) * 512],
                                khT[h * 64:(h + 1) * 64, pr,
                                    kt * 128:(kt + 1) * 128],
                                qhT[h * 64:(h + 1) * 64, pr,
                                    q0:q0 + 512],
                                start=True, stop=True)
                        return sct
                    return scores

                sc_carry = None
                for ui, (qck, pr) in enumerate(units):
                    q0 = qck * 512
                    scores = mk_scores(pr, q0)
                    at = [vap.tile([HD + 1, 512], f32, tag="at",
                                   name=f"at{i}") for i in range(2)]
                    if ui == 0:
                        if 'v' in phases:
                            emit_vchunk(0)
                        sc_cur = scores(0)
                    else:
                        sc_cur = sc_carry

                    for kt in range(KT_ATT):
                        ex = expool.tile([128, 1024], f16, tag="ex")
                        nc.scalar.activation(ex[:], sc_cur[:], ACT.Exp,
                                             scale=SCALE)
                        if kt + 1 < KT_ATT:
                            sc_cur = scores(kt + 1)
                            if ui == 0 and 'v' in phases:
                                emit_vchunk(kt + 1)
                            elif kt >= 8:
                                # drain fin blocks only in the second half of
                                # the kt loop: the producing units' normalize
                                # chains (DRAM-bounce latency) have finished
                                # writing out_sT by then, so the in-order PE
                                # queue never stalls on the fin matmul.
                                emit_fin_block()
                        elif ui + 1 < len(units):
                            nq, npr = units[ui + 1]
                            sc_carry = mk_scores(npr, nq * 512)(0)
                        for h in range(2):
                            nc.tensor.matmul(
                                at[h][:],
                                vh[:, kt, pr * 2 + h, :],
                                ex[:, h * 512:(h + 1) * 512],
                                start=(kt == 0), stop=(kt == KT_ATT - 1),
                                skip_group_check=True)

                    # evacuate + normalize (DVE/SP only; PE moves on)
                    stg = [spool.tile([HD + 1, 512], f32, tag="stg",
                                      name=f"stg{i}", bufs=4)
                           for i in range(2)]
                    rc = spool.tile([HD + 1, 1024], f32, tag="recip")
                    for h in range(2):
                        nc.vector.tensor_copy(stg[h][:], at[h][:])
                        nc.vector.reciprocal(
                            rc[HD:HD + 1, h * 512:(h + 1) * 512],
                            stg[h][HD:HD + 1, :])
                    dnr = dnrpool.tile([1, 1024], f32, tag="dnr")
                    nc.sync.dma_start(dnr[:], rc[HD:HD + 1, :])
                    rb = [spool.tile([64, 512], f32, tag="rb",
                                     name=f"rb{i}", bufs=4)
                          for i in range(2)]
                    for h in range(2):
                        nc.sync.dma_start(
                            rb[h][:],
                            dnr[0:1, h * 512:(h + 1) * 512]
                            .to_broadcast((64, 512)))
                    nc.vector.tensor_mul(
                        out_sT[0:64, pr, q0:q0 + 512],
                        stg[0][0:HD, :], rb[0][:])
                    tmp = spool.tile([64, 512], f16, tag="tmp")
                    nc.vector.tensor_mul(tmp[:], stg[1][0:HD, :], rb[1][:])
                    if 'noshift' not in phases:
                        nc.sync.dma_start(
                            out_sT[64:128, pr, q0:q0 + 512], tmp[:])

                    if pr == PAIRS - 1 and 'fin' in phases:
                        for st in range(4 * qck, 4 * qck + 4):
                            fin_queue.append((st, 0))
                            fin_queue.append((st, 1))

                while fin_queue:
                    emit_fin_block()

            dnrpool.release()
            vap.release()

    nc.finalize()
    if split:
        _split_waits(nc, nop_templates)
    return nc


def _get_program():
    global _PROGRAM
    if _PROGRAM is None:
        _PROGRAM = _build_program()
    return _PROGRAM


def _make_in_maps(q, k, v, Wq, bq, Wk, bk, Wv, bv, Wo, bo):
    q = np.asarray(q, dtype=np.float32)
    k = np.asarray(k, dtype=np.float32)
    v = np.asarray(v, dtype=np.float32)
    Wq = np.asarray(Wq, dtype=np.float32)
    Wk = np.asarray(Wk, dtype=np.float32)
    Wv = np.asarray(Wv, dtype=np.float32)
    Wo = np.asarray(Wo, dtype=np.float32)
    bq = np.asarray(bq, dtype=np.float32)
    bk = np.asarray(bk, dtype=np.float32)
    bv = np.asarray(bv, dtype=np.float32)

    xT = {b: {
        "q": np.ascontiguousarray(q[b].T.astype(np.float16)),
        "k": np.ascontiguousarray(k[b].T.astype(np.float16)),
        "v": np.ascontiguousarray(v[b].T.astype(np.float16)),
    } for b in range(B)}

    in_maps = []
    for c in range(NCORES):
        b = c // GROUPS
        g = c % GROUPS
        hs = g * CS
        in_maps.append({
            "xqT": xT[b]["q"],
            "xkT": xT[b]["k"],
            "xvT": xT[b]["v"],
            "wqT": np.ascontiguousarray(Wq[hs:hs + CS, :].T.astype(np.float16)),
            "wkT": np.ascontiguousarray(Wk[hs:hs + CS, :].T.astype(np.float16)),
            "wvT": np.ascontiguousarray(Wv[hs:hs + CS, :].T.astype(np.float16)),
            "woT": np.ascontiguousarray(Wo[:, hs:hs + CS].T.astype(np.float16)),
            "bq_s": np.ascontiguousarray(bq[hs:hs + CS].reshape(CS, 1)),
            "bk_s": np.ascontiguousarray(bk[hs:hs + CS].reshape(CS, 1)),
            "bv_s": np.ascontiguousarray(bv[hs:hs + CS].reshape(1, CS)),
            "ones_c": np.ones((1, 16 * 4), np.float16),
        })
    return in_maps


def _combine(results, bo):
    bo = np.asarray(bo, dtype=np.float32)
    out = np.zeros((B, S, DIM), np.float32)
    for c in range(NCORES):
        out[c // GROUPS] += results[c]["y"]
    out += bo
    return out


def run_on_hw(inputs, trace=False, **kwargs):
    from concourse.bass_utils import run_bass_kernel_spmd

    nc = _get_program()
    in_maps = _make_in_maps(**inputs)
    res = run_bass_kernel_spmd(nc, in_maps, list(range(NCORES)),
                               trace=trace, **kwargs)
    return _combine(res.results, inputs["bo"]), res


def kernel(**inputs) -> np.ndarray:
    out, _ = run_on_hw(inputs, trace=False)
    return out


# revision 5
# speedup vs baseline: 1.0902x; 1.0902x over previous
"""Multi-head attention (B=2, S=2048, D=1024, H=16) on 8 Trainium2 cores.

Sharding: batch x head-group. Core c handles batch b = c//4 and heads
4*(c%4) .. 4*(c%4)+4 (a 256-wide slice of the feature dim).

Structure (single fused pass per core):
  - Q projection (transposed [d, s] output, f16) chases the xq tile stream
    in an 8-bank PSUM accumulation; weights are batched in single
    rearranged DMAs on the ACT queue so the SP queue only carries x.
  - K projection runs as per-chunk chains (kt innermost, one PSUM bank per
    chain) on the shared 2-bank rotation, so khT chunks become available
    progressively and attention starts right after the xk stream lands.
  - attention per (512-wide q chunk, head pair): scores [k, q] on PE (f16,
    two heads row-packed), exp on ScalarE (the pace-setting engine: the
    exp wall is ~1.4us per [128,1024] tile on this silicon), attn@V
    accumulated in PSUM with a ones-column emitting the softmax
    denominator as row 64; the next unit's first scores are prefetched at
    kt==15 so the exp pipeline never drains at unit boundaries.
  - v-projection chunks are interleaved into attention unit 0's kt loop
    (one chunk per step, emitted a step before the attnV that consumes
    it); output-projection blocks drain into later units' kt>=8 slots
    (after the producer's normalize chain has finished); y written per
    s-tile in single [128,1024] DMAs.
PSUM plan: sc (2 banks x2 bufs) + at (1 bank x2) + chains/vp/yp
(1 bank x2) = 8 banks. All attention operands are f16 (FWL-eligible
matmuls, half SBUF traffic); accumulation stays fp32 in PSUM.
"""
import copy
import sys

import numpy as np

if "/opt/trn_rl_repo" not in sys.path:
    sys.path.insert(0, "/opt/trn_rl_repo")

B = 2
S = 2048
DIM = 1024
NH = 16
HD = 64
NCORES = 8
GROUPS = NCORES // B          # 4 head-groups per batch
HPC = NH // GROUPS            # 4 heads per core
CS = HPC * HD                 # 256-wide feature slice per core
PAIRS = HPC // 2              # head pairs per core


def _split_waits(nc, templates, max_waits=1):
    """This walrus build rejects instructions carrying more than one sync-wait
    command. Move excess waits onto injected same-engine NOPs placed right
    before the over-subscribed instruction."""
    from concourse import mybir

    n_nops = 0
    for f in nc.m.functions:
        for blk in f.blocks:
            insts = blk.instructions
            i = 0
            while i < len(insts):
                inst = insts[i]
                si = inst.sync_info
                if si is not None and si.on_wait and len(si.on_wait) > max_waits:
                    waits = list(si.on_wait)
                    keep = waits[-max_waits:]
                    extra = waits[:-max_waits]
                    nops = []
                    for w in extra:
                        nop = copy.deepcopy(templates[inst.engine])
                        nop.name = f"waitnop-{n_nops}"
                        nop.sync_info = mybir.SyncInfo(on_wait=[w], on_update=[])
                        nops.append(nop)
                        n_nops += 1
                    inst.sync_info = mybir.SyncInfo(
                        on_wait=keep, on_update=list(si.on_update))
                    insts[i:i] = nops
                    i += len(nops)
                i += 1
    return n_nops

_PROGRAM = None


def _build_program(split=True, phases=('qk', 'v', 'att', 'fin'), loop_n=1):
    import concourse.bass as bass
    import concourse.tile as tile
    from concourse import mybir
    import contextlib

    dt = mybir.dt
    f32 = dt.float32
    f16 = dt.float16
    ACT = mybir.ActivationFunctionType

    nc = bass.Bass()
    nop_templates = {
        eng.engine: eng.nop().ins
        for eng in (nc.tensor, nc.vector, nc.scalar, nc.gpsimd, nc.sync)
    }

    xqT = nc.dram_tensor("xqT", [DIM, S], f16, kind="ExternalInput")
    xkT = nc.dram_tensor("xkT", [DIM, S], f16, kind="ExternalInput")
    xvT = nc.dram_tensor("xvT", [DIM, S], f16, kind="ExternalInput")
    wqT = nc.dram_tensor("wqT", [DIM, CS], f16, kind="ExternalInput")
    wkT = nc.dram_tensor("wkT", [DIM, CS], f16, kind="ExternalInput")
    wvT = nc.dram_tensor("wvT", [DIM, CS], f16, kind="ExternalInput")
    woT = nc.dram_tensor("woT", [CS, DIM], f16, kind="ExternalInput")
    bq_s = nc.dram_tensor("bq_s", [CS, 1], f32, kind="ExternalInput")
    bk_s = nc.dram_tensor("bk_s", [CS, 1], f32, kind="ExternalInput")
    bv_s = nc.dram_tensor("bv_s", [1, CS], f32, kind="ExternalInput")
    ones_c = nc.dram_tensor("ones_c", [1, (S // 128) * HPC], f16, kind="ExternalInput")
    y = nc.dram_tensor("y", [S, DIM], f32, kind="ExternalOutput")

    KT_PROJ = DIM // 128
    KT_ATT = S // 128
    ST = S // 128
    SCALE = 1.0 / np.sqrt(np.float32(DIM))

    with tile.TileContext(nc) as tc:
        loop_cm = (
            tc.For_i(0, loop_n, 1, hint_engines=(
                mybir.EngineType.PE, mybir.EngineType.Activation,
                mybir.EngineType.DVE, mybir.EngineType.SP))
            if loop_n > 1 else contextlib.nullcontext())
        with (
            loop_cm,
            tc.tile_pool(name="weights", bufs=1) as wpool,
            tc.tile_pool(name="persist", bufs=1) as persist,
            tc.tile_pool(name="xstream", bufs=4) as xpool,
            tc.tile_pool(name="xvstream", bufs=3) as xvpool,
            tc.tile_pool(name="exp", bufs=3) as expool,
            tc.tile_pool(name="small", bufs=2) as spool,
            tc.tile_pool(name="yout", bufs=2) as ypool,
        ):
            # ---- SBUF allocations ----
            wq_sb = wpool.tile([128, KT_PROJ, CS], f16, tag="wq")
            wk_sb = wpool.tile([128, KT_PROJ, CS], f16, tag="wk")
            wv_sb = wpool.tile([128, KT_PROJ, CS], f16, tag="wv")
            wo_sb = wpool.tile([128, PAIRS, DIM], f16, tag="wo")
            bq_sb = wpool.tile([128, PAIRS], f32, tag="bq")
            bk_sb = wpool.tile([128, PAIRS], f32, tag="bk")
            vb_sb = wpool.tile([128, CS], f32, tag="vb")
            qhT = persist.tile([128, PAIRS, S], f16, tag="qhT")
            khT = persist.tile([128, PAIRS, S], f16, tag="khT")
            vh = persist.tile([128, ST, HPC, HD + 1], f16, tag="vh")
            out_sT = persist.tile([128, PAIRS, S], f16, tag="out_sT")

            # ---- prologue DMAs: weights batched on the ACT queue so the SP
            # queue only carries the x streams; xv late (consumed during att).
            nc.scalar.dma_start(wq_sb[:], wqT.rearrange("(a p) c -> p a c", p=128))
            nc.scalar.dma_start(wk_sb[:], wkT.rearrange("(a p) c -> p a c", p=128))
            for pr in range(PAIRS):
                nc.scalar.dma_start(bq_sb[:, pr:pr + 1], bq_s[pr * 128:(pr + 1) * 128, :])
                nc.scalar.dma_start(bk_sb[:, pr:pr + 1], bk_s[pr * 128:(pr + 1) * 128, :])
            nc.scalar.dma_start(wv_sb[:], wvT.rearrange("(a p) c -> p a c", p=128))
            nc.scalar.dma_start(vb_sb[:], bv_s[:].to_broadcast((128, CS)))
            nc.scalar.dma_start(
                vh[:, :, :, HD:HD + 1],
                ones_c[:].to_broadcast((128, ST * HPC)))
            nc.scalar.dma_start(wo_sb[:], woT.rearrange("(a p) c -> p a c", p=128))

            if 'qk' in phases:
                # ---- Q/K projections (transposed outputs, head-pair layout),
                # matmuls chase the x tiles; dedicated buffers so every x DMA
                # issues immediately ----
                for (xT, w_sb, out_sb, bias_sb, xtag, xbufs) in (
                    (xqT, wq_sb, qhT, bq_sb, "xq", 8),
                ):
                    with tc.tile_pool(name="qkpsum", bufs=2, space="PSUM") as qkp:
                        ps = [qkp.tile([128, S], f32, tag="qk", name=f"qkps{i}") for i in range(PAIRS)]
                        for kt in range(KT_PROJ):
                            xt = xpool.tile([128, S], f16, tag=xtag,
                                            name=f"{xtag}{kt}", bufs=xbufs)
                            nc.sync.dma_start(xt[:], xT[kt * 128:(kt + 1) * 128, :])
                            for pr in range(PAIRS):
                                for qc in range(S // 512):
                                    nc.tensor.matmul(
                                        ps[pr][:, qc * 512:(qc + 1) * 512],
                                        w_sb[:, kt, pr * 128:(pr + 1) * 128],
                                        xt[:, qc * 512:(qc + 1) * 512],
                                        start=(kt == 0), stop=(kt == KT_PROJ - 1),
                                    )
                        for pr in range(PAIRS):
                            nc.scalar.activation(
                                out_sb[:, pr, :], ps[pr][:],
                                ACT.Identity, bias=bias_sb[:, pr:pr + 1], scale=1.0)

            # xk stream: dedicated tiles, DMAs queued on SP behind xq so
            # the q chase keeps full bandwidth first.
            xk_tiles = []
            if 'qk' in phases:
                for kt in range(KT_PROJ):
                    xt = xpool.tile([128, S], f16, tag="xk",
                                    name=f"xk{kt}", bufs=8)
                    nc.sync.dma_start(xt[:], xkT[kt * 128:(kt + 1) * 128, :])
                    xk_tiles.append(xt)

            # ---- V projection + attention + output projection, fully
            # interleaved. One PSUM pool: sc (2 banks x2) + at (1 bank x2) +
            # vp/yp/k-chains (1 bank x2) = 8 banks.
            vap = tc.alloc_tile_pool(name="vattps", bufs=2, space="PSUM")
            dnrpool = tc.alloc_tile_pool(name="dnr", bufs=2, space="DRAM")

            if 'qk' in phases:
                # ---- K projection as per-chunk chains (kt innermost, one
                # PSUM bank per chain, qc-major): khT chunk qc is ready as
                # soon as its two chains evacuate, so attention's first
                # units start ~immediately after the xk stream lands while
                # later chunks still compute under the attention loop.
                for qc in range(S // 512):
                    for pr in range(PAIRS):
                        pp = vap.tile([128, 512], f32, tag="vp",
                                      name="kchain")
                        for kt in range(KT_PROJ):
                            nc.tensor.matmul(
                                pp[:],
                                wk_sb[:, kt, pr * 128:(pr + 1) * 128],
                                xk_tiles[kt][:, qc * 512:(qc + 1) * 512],
                                start=(kt == 0), stop=(kt == KT_PROJ - 1),
                            )
                        nc.scalar.activation(
                            khT[:, pr, qc * 512:(qc + 1) * 512], pp[:],
                            ACT.Identity, bias=bk_sb[:, pr:pr + 1], scale=1.0)

            if 'v' in phases:
                # xv slices stream on the SP queue behind xq/xk; v-projection
                # chunk st is emitted inside attention unit 0's kt loop just
                # before the attnV that consumes it.
                xvT_r = xvT.rearrange("(t p) (s c) -> p t s c", p=128, c=256)
                xv_tiles = []
                for st2 in range(ST // 2):
                    xvt = xvpool.tile([128, KT_PROJ, 256], f16, tag="xv",
                                      name=f"xv{st2}", bufs=8)
                    nc.sync.dma_start(xvt[:], xvT_r[:, :, st2, :])
                    xv_tiles.append(xvt)

                def emit_vchunk(st):
                    st2, sub = st // 2, st % 2
                    vp = vap.tile([128, CS], f32, tag="vp", name="vp")
                    for kt2 in range(KT_PROJ):
                        nc.tensor.matmul(
                            vp[:], xv_tiles[st2][:, kt2, sub * 128:(sub + 1) * 128],
                            wv_sb[:, kt2, :],
                            start=(kt2 == 0), stop=(kt2 == KT_PROJ - 1))
                    nc.vector.tensor_add(
                        vh[:, st, :, 0:HD],
                        vp[:].rearrange("p (h c) -> p h c", c=HD),
                        vb_sb[:].rearrange("p (h c) -> p h c", c=HD))

            if 'att' in phases:
                units = [(qck, pr) for qck in range(S // 512)
                         for pr in range(PAIRS)]
                fin_queue = []          # (st, n2) output-projection blocks
                ysb_cur = {}

                def emit_fin_block():
                    if not fin_queue or 'fin' not in phases:
                        return
                    st, n2 = fin_queue.pop(0)
                    if n2 == 0:
                        ysb_cur[st] = ypool.tile([128, DIM], f32, tag="ysb",
                                                 name=f"ysb{st}")
                    yp = vap.tile([128, 512], f32, tag="vp", name="yp")
                    for pr2 in range(PAIRS):
                        nc.tensor.matmul(
                            yp[:],
                            out_sT[:, pr2, st * 128:(st + 1) * 128],
                            wo_sb[:, pr2, n2 * 512:(n2 + 1) * 512],
                            start=(pr2 == 0), stop=(pr2 == PAIRS - 1))
                    nc.vector.tensor_copy(
                        ysb_cur[st][:, n2 * 512:(n2 + 1) * 512], yp[:])
                    if n2 == 1 and 'noy' not in phases:
                        nc.sync.dma_start(
                            y[st * 128:(st + 1) * 128, :], ysb_cur[st][:])

                def mk_scores(pr, q0):
                    def scores(kt):
                        sct = vap.tile([128, 1024], f32, tag="sc",
                                       name="sct")
                        for h in range(2):
                            nc.tensor.matmul(
                                sct[:, h * 512:(h + 1) * 512],
                                khT[h * 64:(h + 1) * 64, pr,
                                    kt * 128:(kt + 1) * 128],
                                qhT[h * 64:(h + 1) * 64, pr,
                                    q0:q0 + 512],
                                start=True, stop=True)
                        return sct
                    return scores

                sc_carry = None
                for ui, (qck, pr) in enumerate(units):
                    q0 = qck * 512
                    scores = mk_scores(pr, q0)
                    at = [vap.tile([HD + 1, 512], f32, tag="at",
                                   name=f"at{i}") for i in range(2)]
                    if ui == 0:
                        if 'v' in phases:
                            emit_vchunk(0)
                        sc_cur = scores(0)
                    else:
                        sc_cur = sc_carry

                    for kt in range(KT_ATT):
                        ex = expool.tile([128, 1024], f16, tag="ex")
                        nc.scalar.activation(ex[:], sc_cur[:], ACT.Exp,
                                             scale=SCALE)
                        if kt + 1 < KT_ATT:
                            sc_cur = scores(kt + 1)
                            if ui == 0 and 'v' in phases:
                                emit_vchunk(kt + 1)
                            elif kt >= 8:
                                # drain fin blocks only in the second half of
                                # the kt loop: the producing units' normalize
                                # chains (DRAM-bounce latency) have finished
                                # writing out_sT by then, so the in-order PE
                                # queue never stalls on the fin matmul.
                                emit_fin_block()
                        elif ui + 1 < len(units):
                            nq, npr = units[ui + 1]
                            sc_carry = mk_scores(npr, nq * 512)(0)
                        for h in range(2):
                            nc.tensor.matmul(
                                at[h][:],
                                vh[:, kt, pr * 2 + h, :],
                                ex[:, h * 512:(h + 1) * 512],
                                start=(kt == 0), stop=(kt == KT_ATT - 1),
                                skip_group_check=True)

                    # evacuate + normalize (DVE/SP only; PE moves on)
                    stg = [spool.tile([HD + 1, 512], f32, tag="stg",
                                      name=f"stg{i}", bufs=4)
                           for i in range(2)]
                    rc = spool.tile([HD + 1, 1024], f32, tag="recip")
                    for h in range(2):
                        nc.vector.tensor_copy(stg[h][:], at[h][:])
                        nc.vector.reciprocal(
                            rc[HD:HD + 1, h * 512:(h + 1) * 512],
                            stg[h][HD:HD + 1, :])
                    dnr = dnrpool.tile([1, 1024], f32, tag="dnr")
                    nc.sync.dma_start(dnr[:], rc[HD:HD + 1, :])
                    rb = [spool.tile([64, 512], f32, tag="rb",
                                     name=f"rb{i}", bufs=4)
                          for i in range(2)]
                    for h in range(2):
                        nc.sync.dma_start(
                            rb[h][:],
                            dnr[0:1, h * 512:(h + 1) * 512]
                            .to_broadcast((64, 512)))
                    nc.vector.tensor_mul(
                        out_sT[0:64, pr, q0:q0 + 512],
                        stg[0][0:HD, :], rb[0][:])
                    tmp = spool.tile([64, 512], f16, tag="tmp")
                    nc.vector.tensor_mul(tmp[:], stg[1][0:HD, :], rb[1][:])
                    if 'noshift' not in phases:
                        nc.sync.dma_start(
                            out_sT[64:128, pr, q0:q0 + 512], tmp[:])

                    if pr == PAIRS - 1 and 'fin' in phases:
                        for st in range(4 * qck, 4 * qck + 4):
                            fin_queue.append((st, 0))
                            fin_queue.append((st, 1))

                while fin_queue:
                    emit_fin_block()

            dnrpool.release()
            vap.release()

    nc.finalize()
    if split:
        _split_waits(nc, nop_templates)
    return nc


def _get_program():
    global _PROGRAM
    if _PROGRAM is None:
        _PROGRAM = _build_program()
    return _PROGRAM


def _make_in_maps(q, k, v, Wq, bq, Wk, bk, Wv, bv, Wo, bo):
    q = np.asarray(q, dtype=np.float32)
    k = np.asarray(k, dtype=np.float32)
    v = np.asarray(v, dtype=np.float32)
    Wq = np.asarray(Wq, dtype=np.float32)
    Wk = np.asarray(Wk, dtype=np.float32)
    Wv = np.asarray(Wv, dtype=np.float32)
    Wo = np.asarray(Wo, dtype=np.float32)
    bq = np.asarray(bq, dtype=np.float32)
    bk = np.asarray(bk, dtype=np.float32)
    bv = np.asarray(bv, dtype=np.float32)

    xT = {b: {
        "q": np.ascontiguousarray(q[b].T.astype(np.float16)),
        "k": np.ascontiguousarray(k[b].T.astype(np.float16)),
        "v": np.ascontiguousarray(v[b].T.astype(np.float16)),
    } for b in range(B)}

    in_maps = []
    for c in range(NCORES):
        b = c // GROUPS
        g = c % GROUPS
        hs = g * CS
        in_maps.append({
            "xqT": xT[b]["q"],
            "xkT": xT[b]["k"],
            "xvT": xT[b]["v"],
            "wqT": np.ascontiguousarray(Wq[hs:hs + CS, :].T.astype(np.float16)),
            "wkT": np.ascontiguousarray(Wk[hs:hs + CS, :].T.astype(np.float16)),
            "wvT": np.ascontiguousarray(Wv[hs:hs + CS, :].T.astype(np.float16)),
            "woT": np.ascontiguousarray(Wo[:, hs:hs + CS].T.astype(np.float16)),
            "bq_s": np.ascontiguousarray(bq[hs:hs + CS].reshape(CS, 1)),
            "bk_s": np.ascontiguousarray(bk[hs:hs + CS].reshape(CS, 1)),
            "bv_s": np.ascontiguousarray(bv[hs:hs + CS].reshape(1, CS)),
            "ones_c": np.ones((1, 16 * 4), np.float16),
        })
    return in_maps


def _combine(results, bo):
    bo = np.asarray(bo, dtype=np.float32)
    out = np.zeros((B, S, DIM), np.float32)
    for c in range(NCORES):
        out[c // GROUPS] += results[c]["y"]
    out += bo
    return out


def run_on_hw(inputs, trace=False, **kwargs):
    from concourse.bass_utils import run_bass_kernel_spmd

    nc = _get_program()
    in_maps = _make_in_maps(**inputs)
    res = run_bass_kernel_spmd(nc, in_maps, list(range(NCORES)),
                               trace=trace, **kwargs)
    return _combine(res.results, inputs["bo"]), res


def kernel(**inputs) -> np.ndarray:
    out, _ = run_on_hw(inputs, trace=False)
    return out


# revision 9
# speedup vs baseline: 1.1046x; 1.0132x over previous
"""Multi-head attention (B=2, S=2048, D=1024, H=16) on 8 Trainium2 cores.

Sharding: batch x head-group. Core c handles batch b = c//4 and heads
4*(c%4) .. 4*(c%4)+4 (a 256-wide slice of the feature dim).

Structure (single fused pass per core):
  - Q projection (transposed [d, s] output, f16) chases the xq tile stream
    in an 8-bank PSUM accumulation; weights are batched in single
    rearranged DMAs on the ACT queue so the SP queue only carries x.
  - K projection runs as per-chunk chains (kt innermost, one PSUM bank per
    chain) on the shared 2-bank rotation, so khT chunks become available
    progressively and attention starts right after the xk stream lands.
  - attention per (512-wide q chunk, head pair): scores [k, q] on PE (f16,
    two heads row-packed), exp on ScalarE (the pace-setting engine: the
    exp wall is ~1.4us per [128,1024] tile on this silicon), attn@V
    accumulated in PSUM with a ones-column emitting the softmax
    denominator as row 64; the next unit's first scores are prefetched at
    kt==15 so the exp pipeline never drains at unit boundaries.
  - v-projection chunks are interleaved into attention unit 0's kt loop
    (one chunk per step, emitted a step before the attnV that consumes
    it); output-projection blocks drain into later units' kt>=8 slots
    (after the producer's normalize chain has finished); y written per
    s-tile in single [128,1024] DMAs.
PSUM plan: sc (2 banks x2 bufs) + at (1 bank x2) + chains/vp/yp
(1 bank x2) = 8 banks. All attention operands are f16 (FWL-eligible
matmuls, half SBUF traffic); accumulation stays fp32 in PSUM.
"""
import copy
import sys

import numpy as np

if "/opt/trn_rl_repo" not in sys.path:
    sys.path.insert(0, "/opt/trn_rl_repo")

B = 2
S = 2048
DIM = 1024
NH = 16
HD = 64
NCORES = 8
GROUPS = NCORES // B          # 4 head-groups per batch
HPC = NH // GROUPS            # 4 heads per core
CS = HPC * HD                 # 256-wide feature slice per core
PAIRS = HPC // 2              # head pairs per core


def _split_waits(nc, templates, max_waits=1):
    """This walrus build rejects instructions carrying more than one sync-wait
    command. Move excess waits onto injected same-engine NOPs placed right
    before the over-subscribed instruction."""
    from concourse import mybir

    n_nops = 0
    for f in nc.m.functions:
        for blk in f.blocks:
            insts = blk.instructions
            i = 0
            while i < len(insts):
                inst = insts[i]
                si = inst.sync_info
                if si is not None and si.on_wait and len(si.on_wait) > max_waits:
                    waits = list(si.on_wait)
                    keep = waits[-max_waits:]
                    extra = waits[:-max_waits]
                    nops = []
                    for w in extra:
                        nop = copy.deepcopy(templates[inst.engine])
                        nop.name = f"waitnop-{n_nops}"
                        nop.sync_info = mybir.SyncInfo(on_wait=[w], on_update=[])
                        nops.append(nop)
                        n_nops += 1
                    inst.sync_info = mybir.SyncInfo(
                        on_wait=keep, on_update=list(si.on_update))
                    insts[i:i] = nops
                    i += len(nops)
                i += 1
    return n_nops

_PROGRAM = None


def _build_program(split=True, phases=('qk', 'v', 'att', 'fin'), loop_n=1):
    import concourse.bass as bass
    import concourse.tile as tile
    from concourse import mybir
    import contextlib

    dt = mybir.dt
    f32 = dt.float32
    f16 = dt.float16
    ACT = mybir.ActivationFunctionType

    nc = bass.Bass()
    nop_templates = {
        eng.engine: eng.nop().ins
        for eng in (nc.tensor, nc.vector, nc.scalar, nc.gpsimd, nc.sync)
    }

    xqT = nc.dram_tensor("xqT", [DIM, S], f16, kind="ExternalInput")
    xkT = nc.dram_tensor("xkT", [DIM, S], f16, kind="ExternalInput")
    xvT = nc.dram_tensor("xvT", [DIM, S], f16, kind="ExternalInput")
    wqT = nc.dram_tensor("wqT", [DIM, CS], f16, kind="ExternalInput")
    wkT = nc.dram_tensor("wkT", [DIM, CS], f16, kind="ExternalInput")
    wvT = nc.dram_tensor("wvT", [DIM, CS], f16, kind="ExternalInput")
    woT = nc.dram_tensor("woT", [CS, DIM], f16, kind="ExternalInput")
    bq_s = nc.dram_tensor("bq_s", [CS, 1], f32, kind="ExternalInput")
    bk_s = nc.dram_tensor("bk_s", [CS, 1], f32, kind="ExternalInput")
    bv_s = nc.dram_tensor("bv_s", [1, CS], f32, kind="ExternalInput")
    ones_c = nc.dram_tensor("ones_c", [1, (S // 128) * HPC], f16, kind="ExternalInput")
    y = nc.dram_tensor("y", [S, DIM], f32, kind="ExternalOutput")

    KT_PROJ = DIM // 128
    KT_ATT = S // 128
    ST = S // 128
    SCALE = 1.0 / np.sqrt(np.float32(DIM))

    with tile.TileContext(nc) as tc:
        loop_cm = (
            tc.For_i(0, loop_n, 1, hint_engines=(
                mybir.EngineType.PE, mybir.EngineType.Activation,
                mybir.EngineType.DVE, mybir.EngineType.SP))
            if loop_n > 1 else contextlib.nullcontext())
        with (
            loop_cm,
            tc.tile_pool(name="weights", bufs=1) as wpool,
            tc.tile_pool(name="persist", bufs=1) as persist,
            tc.tile_pool(name="xstream", bufs=4) as xpool,
            tc.tile_pool(name="xvstream", bufs=3) as xvpool,
            tc.tile_pool(name="exp", bufs=3) as expool,
            tc.tile_pool(name="small", bufs=2) as spool,
            tc.tile_pool(name="yout", bufs=2) as ypool,
        ):
            # ---- SBUF allocations ----
            wq_sb = wpool.tile([128, KT_PROJ, CS], f16, tag="wq")
            wk_sb = wpool.tile([128, KT_PROJ, CS], f16, tag="wk")
            wv_sb = wpool.tile([128, KT_PROJ, CS], f16, tag="wv")
            wo_sb = wpool.tile([128, PAIRS, DIM], f16, tag="wo")
            bq_sb = wpool.tile([128, PAIRS], f32, tag="bq")
            bk_sb = wpool.tile([128, PAIRS], f32, tag="bk")
            vb_sb = wpool.tile([128, CS], f32, tag="vb")
            qhT = persist.tile([128, PAIRS, S], f16, tag="qhT")
            khT = persist.tile([128, PAIRS, S], f16, tag="khT")
            vh = persist.tile([128, ST, HPC, HD + 1], f16, tag="vh")
            out_sT = persist.tile([128, PAIRS, S], f16, tag="out_sT")

            # ---- prologue DMAs: weights batched on the ACT queue so the SP
            # queue only carries the x streams; xv late (consumed during att).
            nc.scalar.dma_start(wq_sb[:], wqT.rearrange("(a p) c -> p a c", p=128))
            nc.scalar.dma_start(wk_sb[:], wkT.rearrange("(a p) c -> p a c", p=128))
            for pr in range(PAIRS):
                nc.scalar.dma_start(bq_sb[:, pr:pr + 1], bq_s[pr * 128:(pr + 1) * 128, :])
                nc.scalar.dma_start(bk_sb[:, pr:pr + 1], bk_s[pr * 128:(pr + 1) * 128, :])
            nc.scalar.dma_start(wv_sb[:], wvT.rearrange("(a p) c -> p a c", p=128))
            nc.scalar.dma_start(vb_sb[:], bv_s[:].to_broadcast((128, CS)))
            nc.scalar.dma_start(
                vh[:, :, :, HD:HD + 1],
                ones_c[:].to_broadcast((128, ST * HPC)))
            nc.scalar.dma_start(wo_sb[:], woT.rearrange("(a p) c -> p a c", p=128))

            if 'qk' in phases:
                # ---- Q/K projections (transposed outputs, head-pair layout),
                # matmuls chase the x tiles; dedicated buffers so every x DMA
                # issues immediately ----
                for (xT, w_sb, out_sb, bias_sb, xtag, xbufs) in (
                    (xqT, wq_sb, qhT, bq_sb, "xq", 8),
                ):
                    with tc.tile_pool(name="qkpsum", bufs=2, space="PSUM") as qkp:
                        ps = [qkp.tile([128, S], f32, tag="qk", name=f"qkps{i}") for i in range(PAIRS)]
                        for kt in range(KT_PROJ):
                            xt = xpool.tile([128, S], f16, tag=xtag,
                                            name=f"{xtag}{kt}", bufs=xbufs)
                            nc.sync.dma_start(xt[:], xT[kt * 128:(kt + 1) * 128, :])
                            for pr in range(PAIRS):
                                for qc in range(S // 512):
                                    nc.tensor.matmul(
                                        ps[pr][:, qc * 512:(qc + 1) * 512],
                                        w_sb[:, kt, pr * 128:(pr + 1) * 128],
                                        xt[:, qc * 512:(qc + 1) * 512],
                                        start=(kt == 0), stop=(kt == KT_PROJ - 1),
                                    )
                        for pr in range(PAIRS):
                            nc.scalar.activation(
                                out_sb[:, pr, :], ps[pr][:],
                                ACT.Identity, bias=bias_sb[:, pr:pr + 1], scale=1.0)

            # xk stream: dedicated tiles, DMAs queued on SP behind xq so
            # the q chase keeps full bandwidth first.
            xk_tiles = []
            if 'qk' in phases:
                for kt in range(KT_PROJ):
                    xt = xpool.tile([128, S], f16, tag="xk",
                                    name=f"xk{kt}", bufs=8)
                    nc.sync.dma_start(xt[:], xkT[kt * 128:(kt + 1) * 128, :])
                    xk_tiles.append(xt)

            # ---- V projection + attention + output projection, fully
            # interleaved. One PSUM pool: sc (2 banks x2) + at (1 bank x2) +
            # vp/yp/k-chains (1 bank x2) = 8 banks.
            vap = tc.alloc_tile_pool(name="vattps", bufs=2, space="PSUM")
            dnrpool = tc.alloc_tile_pool(name="dnr", bufs=2, space="DRAM")

            if 'qk' in phases:
                # ---- K projection as per-chunk chains (kt innermost, one
                # PSUM bank per chain, qc-major): khT chunk qc is ready as
                # soon as its two chains evacuate, so attention's first
                # units start ~immediately after the xk stream lands while
                # later chunks still compute under the attention loop.
                for qc in range(S // 512):
                    for pr in range(PAIRS):
                        pp = vap.tile([128, 512], f32, tag="vp",
                                      name="kchain")
                        for kt in range(KT_PROJ):
                            nc.tensor.matmul(
                                pp[:],
                                wk_sb[:, kt, pr * 128:(pr + 1) * 128],
                                xk_tiles[kt][:, qc * 512:(qc + 1) * 512],
                                start=(kt == 0), stop=(kt == KT_PROJ - 1),
                            )
                        nc.scalar.activation(
                            khT[:, pr, qc * 512:(qc + 1) * 512], pp[:],
                            ACT.Identity, bias=bk_sb[:, pr:pr + 1], scale=1.0)

            if 'v' in phases:
                # xv slices stream on the SP queue behind xq/xk; v-projection
                # chunk st is emitted inside attention unit 0's kt loop just
                # before the attnV that consumes it.
                xvT_r = xvT.rearrange("(t p) (s c) -> p t s c", p=128, c=256)
                xv_tiles = []
                for st2 in range(ST // 2):
                    xvt = xvpool.tile([128, KT_PROJ, 256], f16, tag="xv",
                                      name=f"xv{st2}", bufs=8)
                    nc.sync.dma_start(xvt[:], xvT_r[:, :, st2, :])
                    xv_tiles.append(xvt)

                def emit_vchunk(st):
                    st2, sub = st // 2, st % 2
                    vp = vap.tile([128, CS], f32, tag="vp", name="vp")
                    for kt2 in range(KT_PROJ):
                        nc.tensor.matmul(
                            vp[:], xv_tiles[st2][:, kt2, sub * 128:(sub + 1) * 128],
                            wv_sb[:, kt2, :],
                            start=(kt2 == 0), stop=(kt2 == KT_PROJ - 1))
                    nc.vector.tensor_add(
                        vh[:, st, :, 0:HD],
                        vp[:].rearrange("p (h c) -> p h c", c=HD),
                        vb_sb[:].rearrange("p (h c) -> p h c", c=HD))

            if 'att' in phases:
                units = [(qck, pr) for qck in range(S // 512)
                         for pr in range(PAIRS)]
                fin_queue = []          # (st, n2) output-projection blocks
                ysb_cur = {}

                def emit_fin_block():
                    if not fin_queue or 'fin' not in phases:
                        return
                    st, n2 = fin_queue.pop(0)
                    if n2 == 0:
                        ysb_cur[st] = ypool.tile([128, DIM], f32, tag="ysb",
                                                 name=f"ysb{st}")
                    yp = vap.tile([128, 512], f32, tag="vp", name="yp")
                    for pr2 in range(PAIRS):
                        nc.tensor.matmul(
                            yp[:],
                            out_sT[:, pr2, st * 128:(st + 1) * 128],
                            wo_sb[:, pr2, n2 * 512:(n2 + 1) * 512],
                            start=(pr2 == 0), stop=(pr2 == PAIRS - 1))
                    nc.vector.tensor_copy(
                        ysb_cur[st][:, n2 * 512:(n2 + 1) * 512], yp[:])
                    if n2 == 1 and 'noy' not in phases:
                        nc.sync.dma_start(
                            y[st * 128:(st + 1) * 128, :], ysb_cur[st][:])

                def mk_scores(pr, q0):
                    def scores(kt):
                        sct = vap.tile([128, 1024], f32, tag="sc",
                                       name="sct")
                        for h in range(2):
                            nc.tensor.matmul(
                                sct[:, h * 512:(h + 1) * 512],
                                khT[h * 64:(h + 1) * 64, pr,
                                    kt * 128:(kt + 1) * 128],
                                qhT[h * 64:(h + 1) * 64, pr,
                                    q0:q0 + 512],
                                start=True, stop=True)
                        return sct
                    return scores

                sc_carry = None
                for ui, (qck, pr) in enumerate(units):
                    q0 = qck * 512
                    scores = mk_scores(pr, q0)
                    at = [vap.tile([HD + 1, 512], f32, tag="at",
                                   name=f"at{i}") for i in range(2)]
                    if ui == 0:
                        if 'v' in phases:
                            emit_vchunk(0)
                        sc_cur = scores(0)
                    else:
                        sc_cur = sc_carry

                    for kt in range(KT_ATT):
                        ex = expool.tile([128, 1024], f16, tag="ex")
                        nc.scalar.activation(ex[:], sc_cur[:], ACT.Exp,
                                             scale=SCALE)
                        if kt + 1 < KT_ATT:
                            sc_cur = scores(kt + 1)
                            if ui == 0 and 'v' in phases:
                                emit_vchunk(kt + 1)
                            elif kt >= 8:
                                # drain fin blocks only in the second half of
                                # the kt loop: the producing units' normalize
                                # chains (DRAM-bounce latency) have finished
                                # writing out_sT by then, so the in-order PE
                                # queue never stalls on the fin matmul.
                                emit_fin_block()
                        elif ui + 1 < len(units):
                            nq, npr = units[ui + 1]
                            sc_carry = mk_scores(npr, nq * 512)(0)
                        for h in range(2):
                            nc.tensor.matmul(
                                at[h][:],
                                vh[:, kt, pr * 2 + h, :],
                                ex[:, h * 512:(h + 1) * 512],
                                start=(kt == 0), stop=(kt == KT_ATT - 1),
                                skip_group_check=True)

                    # evacuate + normalize (DVE/SP only; PE moves on)
                    stg = [spool.tile([HD + 1, 512], f32, tag="stg",
                                      name=f"stg{i}", bufs=4)
                           for i in range(2)]
                    rc = spool.tile([HD + 1, 1024], f32, tag="recip")
                    for h in range(2):
                        nc.vector.tensor_copy(stg[h][:], at[h][:])
                        nc.vector.reciprocal(
                            rc[HD:HD + 1, h * 512:(h + 1) * 512],
                            stg[h][HD:HD + 1, :])
                    dnr = dnrpool.tile([1, 1024], f32, tag="dnr")
                    nc.sync.dma_start(dnr[:], rc[HD:HD + 1, :])
                    rb = [spool.tile([64, 512], f32, tag="rb",
                                     name=f"rb{i}", bufs=4)
                          for i in range(2)]
                    for h in range(2):
                        nc.sync.dma_start(
                            rb[h][:],
                            dnr[0:1, h * 512:(h + 1) * 512]
                            .to_broadcast((64, 512)))
                    nc.vector.tensor_mul(
                        out_sT[0:64, pr, q0:q0 + 512],
                        stg[0][0:HD, :], rb[0][:])
                    tmp = spool.tile([64, 512], f16, tag="tmp")
                    nc.vector.tensor_mul(tmp[:], stg[1][0:HD, :], rb[1][:])
                    if 'noshift' not in phases:
                        nc.sync.dma_start(
                            out_sT[64:128, pr, q0:q0 + 512], tmp[:])

                    if pr == PAIRS - 1 and 'fin' in phases:
                        for st in range(4 * qck, 4 * qck + 4):
                            fin_queue.append((st, 0))
                            fin_queue.append((st, 1))

                while fin_queue:
                    emit_fin_block()

            dnrpool.release()
            vap.release()

    nc.finalize()
    if split:
        _split_waits(nc, nop_templates)
    return nc


def _get_program():
    global _PROGRAM
    if _PROGRAM is None:
        _PROGRAM = _build_program()
    return _PROGRAM


def _make_in_maps(q, k, v, Wq, bq, Wk, bk, Wv, bv, Wo, bo):
    q = np.asarray(q, dtype=np.float32)
    k = np.asarray(k, dtype=np.float32)
    v = np.asarray(v, dtype=np.float32)
    Wq = np.asarray(Wq, dtype=np.float32)
    Wk = np.asarray(Wk, dtype=np.float32)
    Wv = np.asarray(Wv, dtype=np.float32)
    Wo = np.asarray(Wo, dtype=np.float32)
    bq = np.asarray(bq, dtype=np.float32)
    bk = np.asarray(bk, dtype=np.float32)
    bv = np.asarray(bv, dtype=np.float32)

    xT = {b: {
        "q": np.ascontiguousarray(q[b].T.astype(np.float16)),
        "k": np.ascontiguousarray(k[b].T.astype(np.float16)),
        "v": np.ascontiguousarray(v[b].T.astype(np.float16)),
    } for b in range(B)}

    in_maps = []
    for c in range(NCORES):
        b = c // GROUPS
        g = c % GROUPS
        hs = g * CS
        in_maps.append({
            "xqT": xT[b]["q"],
            "xkT": xT[b]["k"],
            "xvT": xT[b]["v"],
            "wqT": np.ascontiguousarray(Wq[hs:hs + CS, :].T.astype(np.float16)),
            "wkT": np.ascontiguousarray(Wk[hs:hs + CS, :].T.astype(np.float16)),
            "wvT": np.ascontiguousarray(Wv[hs:hs + CS, :].T.astype(np.float16)),
            "woT": np.ascontiguousarray(Wo[:, hs:hs + CS].T.astype(np.float16)),
            "bq_s": np.ascontiguousarray(bq[hs:hs + CS].reshape(CS, 1)),
            "bk_s": np.ascontiguousarray(bk[hs:hs + CS].reshape(CS, 1)),
            "bv_s": np.ascontiguousarray(bv[hs:hs + CS].reshape(1, CS)),
            "ones_c": np.ones((1, 16 * 4), np.float16),
        })
    return in_maps


def _combine(results, bo):
    bo = np.asarray(bo, dtype=np.float32)
    out = np.zeros((B, S, DIM), np.float32)
    for c in range(NCORES):
        out[c // GROUPS] += results[c]["y"]
    out += bo
    return out


def run_on_hw(inputs, trace=False, **kwargs):
    from concourse.bass_utils import run_bass_kernel_spmd

    nc = _get_program()
    in_maps = _make_in_maps(**inputs)
    res = run_bass_kernel_spmd(nc, in_maps, list(range(NCORES)),
                               trace=trace, **kwargs)
    return _combine(res.results, inputs["bo"]), res


def kernel(**inputs) -> np.ndarray:
    out, _ = run_on_hw(inputs, trace=False)
    return out
